# revision 1
# baseline (speedup 1.0000x reference)
"""Trainium2 Bass kernel for nn_DroneRelationModel (8 NeuronCores).

Strategy:
  - Attention sharded (head, query-half) across 8 cores; scores computed
    transposed in PSUM (row-packed K=32 matmuls), ACT exp, AV matmul with a
    ones-column producing softmax denominators, normalize.
  - AllGather context pieces; every core folds out_proj+w1 into per-node
    tables A,B (|w2|-scaled, sign-permuted hidden, biases folded in).
  - Pair head sharded by pair slices: dma_gather rows of A,B per pair
    (natural layout), add + relu on DVE, sign-split segmented reduce gives
    w2-dot, sigmoid via tanh.
All heavy matmul inputs in bf16 (validated ~0.2% max rel err end to end).
"""
import numpy as np
import ml_dtypes

N, H, HEADS, P = 4096, 128, 4, 262144
DH = 32
NCORES = 8
NQ = 2048
QB = 1024
PC = P // NCORES
GCHUNK = 4096
NGC = PC // GCHUNK
RREP = 3


def _build(npos, b2val):
    import concourse.bass as bass
    import concourse.mybir as mybir
    import concourse.tile as tile
    from concourse import bacc

    F32 = mybir.dt.float32
    BF16 = mybir.dt.bfloat16
    I16 = mybir.dt.int16
    AF = mybir.ActivationFunctionType
    ALU = mybir.AluOpType

    nc = bacc.Bacc("TRN2", target_bir_lowering=False, debug=False, num_devices=NCORES)

    xT = nc.dram_tensor("xT", [H, N], BF16, kind="ExternalInput")
    xTq = nc.dram_tensor("xTq", [H, NQ], BF16, kind="ExternalInput")
    wq_t = nc.dram_tensor("wq_t", [H, DH * RREP], BF16, kind="ExternalInput")
    wk_t = nc.dram_tensor("wk_t", [H, DH * RREP], BF16, kind="ExternalInput")
    wv_t = nc.dram_tensor("wv_t", [H, DH], BF16, kind="ExternalInput")
    bq = nc.dram_tensor("bq", [DH * RREP, 1], F32, kind="ExternalInput")
    wa_t = nc.dram_tensor("wa_t", [H, H], BF16, kind="ExternalInput")
    wb_t = nc.dram_tensor("wb_t", [H, H], BF16, kind="ExternalInput")
    bias_r = nc.dram_tensor("bias_r", [1, H], BF16, kind="ExternalInput")
    srcw = nc.dram_tensor("srcw", [128, PC // 16], I16, kind="ExternalInput")
    dstw = nc.dram_tensor("dstw", [128, PC // 16], I16, kind="ExternalInput")
    preds = nc.dram_tensor("preds", [128, PC // 128], F32, kind="ExternalOutput")
    cc_in = nc.dram_tensor("cc_in", [DH, NQ], BF16)
    cc_out = nc.dram_tensor("cc_out", [NCORES, DH, NQ], BF16, addr_space="Shared")

    with tile.TileContext(nc) as tc:
        with tc.tile_pool(name="const", bufs=1) as cpool:
            def cload(name, dram, shape, dtype):
                t = cpool.tile(shape, dtype, tag=name)
                nc.sync.dma_start(out=t[...], in_=dram[...])
                return t

            xT_sb = cload("xT", xT, [H, N], BF16)
            xTq_sb = cload("xTq", xTq, [H, NQ], BF16)
            wq_sb = cload("wq", wq_t, [H, DH * RREP], BF16)
            wk_sb = cload("wk", wk_t, [H, DH * RREP], BF16)
            wv_sb = cload("wv", wv_t, [H, DH], BF16)
            bq_sb = cload("bq", bq, [DH * RREP, 1], F32)
            wa_sb = cload("wa", wa_t, [H, H], BF16)
            wb_sb = cload("wb", wb_t, [H, H], BF16)
            bias_sb = cload("bias", bias_r, [1, H], BF16)
            src_sb = cload("src", srcw, [128, PC // 16], I16)
            dst_sb = cload("dst", dstw, [128, PC // 16], I16)

            kT_sb = cpool.tile([DH * RREP, N], BF16, tag="kT")
            qT_sb = cpool.tile([DH * RREP, NQ], BF16, tag="qT")
            v_sb = cpool.tile([128, 33 * 32], BF16, tag="v")
            ctxn_sb = cpool.tile([DH, NQ], BF16, tag="ctxn")
            ones_sb = cpool.tile([1, H], BF16, tag="ones")
            nc.vector.memset(ones_sb[...], 1.0)

            with (
                tc.tile_pool(name="kqv_ps", bufs=2, space="PSUM") as kqv_ps,
                tc.tile_pool(name="v_ps", bufs=4, space="PSUM") as v_ps,
            ):
                for i in range(4):
                    ps = kqv_ps.tile([DH * RREP, QB], F32, tag="kq")
                    for j in range(2):
                        nc.tensor.matmul(ps[:, j * 512:(j + 1) * 512], wk_sb[...],
                                         xT_sb[:, i * QB + j * 512:i * QB + (j + 1) * 512],
                                         start=True, stop=True)
                    nc.vector.tensor_copy(kT_sb[:, i * QB:(i + 1) * QB], ps[...])
                for i in range(2):
                    ps = kqv_ps.tile([DH * RREP, QB], F32, tag="kq")
                    for j in range(2):
                        nc.tensor.matmul(ps[:, j * 512:(j + 1) * 512], wq_sb[...],
                                         xTq_sb[:, i * QB + j * 512:i * QB + (j + 1) * 512],
                                         start=True, stop=True)
                    nc.vector.tensor_scalar_add(qT_sb[:, i * QB:(i + 1) * QB],
                                                ps[...], bq_sb[...])
                nc.vector.memset(v_sb[...], 1.0)
                for kc in range(32):
                    ps = v_ps.tile([128, DH], F32, tag="v")
                    nc.tensor.matmul(ps[...], xT_sb[:, kc * 128:(kc + 1) * 128],
                                     wv_sb[...], start=True, stop=True)
                    nc.vector.tensor_copy(v_sb[:, kc * 33:kc * 33 + 32], ps[...])

            with (
                tc.tile_pool(name="s_ps", bufs=3, space="PSUM") as s_ps,
                tc.tile_pool(name="av_ps", bufs=1, space="PSUM") as av_psp,
                tc.tile_pool(name="es", bufs=6) as es_pool,
                tc.tile_pool(name="misc", bufs=2) as misc,
            ):
                for qb in range(2):
                    av_ps = av_psp.tile([128, QB], F32, tag="av")
                    es_tiles = {}
                    for kc in range(32):
                        r = kc % RREP
                        sp = s_ps.tile([128, QB], F32, tag="s")
                        for j in range(2):
                            nc.tensor.matmul(
                                sp[:, j * 512:(j + 1) * 512],
                                kT_sb[r * DH:(r + 1) * DH, kc * 128:(kc + 1) * 128],
                                qT_sb[r * DH:(r + 1) * DH,
                                      qb * QB + j * 512:qb * QB + (j + 1) * 512],
                                start=True, stop=True, tile_position=(r * DH, 0))
                        es = es_pool.tile([128, QB], BF16, tag="es")
                        nc.scalar.activation(es[...], sp[...], AF.Exp)
                        es_tiles[kc] = es
                        if kc % 2 == 1:
                            for t, kk in ((0, kc - 1), (1, kc)):
                                for j in range(2):
                                    nc.tensor.matmul(
                                        av_ps[t * 64:t * 64 + 33, j * 512:(j + 1) * 512],
                                        v_sb[:, kk * 33:(kk + 1) * 33],
                                        es_tiles[kk][:, j * 512:(j + 1) * 512],
                                        start=(kk < 2), stop=(kk >= 30),
                                        tile_position=(0, t * 64))
                            es_tiles.clear()
                    craw = misc.tile([33, QB], F32, tag="craw")
                    t1c = misc.tile([33, QB], F32, tag="t1c")
                    nc.vector.tensor_copy(t1c[...], av_ps[64:97, :])
                    nc.vector.tensor_add(craw[...], av_ps[0:33, :], t1c[...])
                    r_sb = misc.tile([1, QB], F32, tag="r")
                    nc.vector.reciprocal(r_sb[...], craw[32:33, :])
                    rbf = misc.tile([1, QB], BF16, tag="rbf")
                    nc.vector.tensor_copy(rbf[...], r_sb[...])
                    bc_ps = s_ps.tile([DH, QB], F32, tag="s")
                    for j in range(2):
                        nc.tensor.matmul(bc_ps[:, j * 512:(j + 1) * 512],
                                         ones_sb[:, 0:DH], rbf[:, j * 512:(j + 1) * 512],
                                         start=True, stop=True)
                    nc.vector.tensor_mul(ctxn_sb[:, qb * QB:(qb + 1) * QB],
                                         craw[0:32, :], bc_ps[...])

            nc.sync.dma_start(out=cc_in[...], in_=ctxn_sb[...])
            nc.gpsimd.collective_compute(
                "AllGather", ALU.bypass, replica_groups=[list(range(NCORES))],
                ins=[cc_in.ap()], outs=[cc_out.ap()])
            ctxT_sb = cpool.tile([H, N], BF16, tag="ctxT")
            for g in range(NCORES):
                hh, half = g // 2, g % 2
                nc.sync.dma_start(
                    out=ctxT_sb[hh * DH:(hh + 1) * DH, half * NQ:(half + 1) * NQ],
                    in_=cc_out[g, :, :])

            atab = cpool.tile([128, 32 * 128], BF16, tag="atab")
            btab = cpool.tile([128, 32 * 128], BF16, tag="btab")
            with tc.tile_pool(name="tab_ps", bufs=2, space="PSUM") as tab_ps:
                for t in range(32):
                    pa = tab_ps.tile([128, 128], F32, tag="ta")
                    pb = tab_ps.tile([128, 128], F32, tag="tb")
                    nc.tensor.matmul(pa[...], ones_sb[...], bias_sb[...],
                                     start=True, stop=False)
                    nc.tensor.matmul(pa[...], ctxT_sb[:, t * 128:(t + 1) * 128],
                                     wa_sb[...], start=False, stop=True)
                    nc.tensor.matmul(pb[...], ctxT_sb[:, t * 128:(t + 1) * 128],
                                     wb_sb[...], start=True, stop=True)
                    nc.vector.tensor_copy(atab[:, t * 128:(t + 1) * 128], pa[...])
                    nc.vector.tensor_copy(btab[:, t * 128:(t + 1) * 128], pb[...])
            atab_d = nc.dram_tensor("atab_d", [N, H], BF16)
            btab_d = nc.dram_tensor("btab_d", [N, H], BF16)
            nc.sync.dma_start(out=atab_d.ap().rearrange("(c p) d -> p c d", p=128),
                              in_=atab[...].rearrange("p (c d) -> p c d", d=128))
            nc.sync.dma_start(out=btab_d.ap().rearrange("(c p) d -> p c d", p=128),
                              in_=btab[...].rearrange("p (c d) -> p c d", d=128))

            with (
                tc.tile_pool(name="gat", bufs=2) as gpool,
                tc.tile_pool(name="hseq", bufs=2) as hpool,
                tc.tile_pool(name="sig", bufs=1) as sig_pool,
            ):
                NB = GCHUNK // 128
                lsp = sig_pool.tile([128, PC // 128], F32, tag="lsp")
                lsn = sig_pool.tile([128, PC // 128], F32, tag="lsn")
                iw = GCHUNK // 16
                for g in range(NGC):
                    sg = gpool.tile([128, NB, 128], BF16, tag="sg")
                    dg = gpool.tile([128, NB, 128], BF16, tag="dg")
                    nc.gpsimd.dma_gather(
                        sg[...], atab_d[...], src_sb[:, g * iw:(g + 1) * iw],
                        num_idxs=GCHUNK, num_idxs_reg=GCHUNK, elem_size=128,
                        transpose=False, single_packet=False)
                    nc.gpsimd.dma_gather(
                        dg[...], btab_d[...], dst_sb[:, g * iw:(g + 1) * iw],
                        num_idxs=GCHUNK, num_idxs_reg=GCHUNK, elem_size=128,
                        transpose=False, single_packet=False)
                    hs = hpool.tile([128, NB, 128], BF16, tag="hs")
                    nc.vector.tensor_add(hs[...].rearrange("p b d -> p (b d)"),
                                         sg[...].rearrange("p b d -> p (b d)"),
                                         dg[...].rearrange("p b d -> p (b d)"))
                    hr = hpool.tile([128, NB, 128], BF16, tag="hr")
                    nc.vector.tensor_scalar_max(hr[...].rearrange("p b d -> p (b d)"),
                                                hs[...].rearrange("p b d -> p (b d)"), 0.0)
                    nc.vector.tensor_reduce(
                        lsp[:, g * NB:(g + 1) * NB].rearrange("p b -> p b ()"),
                        hr[:, :, 0:npos], op=ALU.add, axis=mybir.AxisListType.X)
                    nc.vector.tensor_reduce(
                        lsn[:, g * NB:(g + 1) * NB].rearrange("p b -> p b ()"),
                        hr[:, :, npos:H], op=ALU.add, axis=mybir.AxisListType.X)
                logit = sig_pool.tile([128, PC // 128], F32, tag="logit")
                nc.vector.tensor_sub(logit[...], lsp[...], lsn[...])
                sig_sb = sig_pool.tile([128, PC // 128], F32, tag="sig")
                nc.scalar.activation(sig_sb[...], logit[...], AF.Tanh,
                                     scale=0.5, bias=0.5 * b2val)
                nc.vector.tensor_scalar(sig_sb[...], sig_sb[...], 0.5, 0.5,
                                        op0=ALU.mult, op1=ALU.add)
                nc.sync.dma_start(out=preds[...], in_=sig_sb[...])
    nc.compile()
    return nc


def _prep_inputs(inputs):
    x = np.asarray(inputs["context_embeddings"], np.float32)
    ipw = np.asarray(inputs["in_proj_w"], np.float32)
    ipb = np.asarray(inputs["in_proj_b"], np.float32)
    opw = np.asarray(inputs["out_proj_w"], np.float32)
    opb = np.asarray(inputs["out_proj_b"], np.float32)
    w1 = np.asarray(inputs["w1"], np.float32)
    b1 = np.asarray(inputs["b1"], np.float32)
    w2v = np.asarray(inputs["w2"], np.float32)[0]
    rel = np.asarray(inputs["relationships"])

    bf = ml_dtypes.bfloat16
    scale = 1.0 / np.sqrt(DH)
    xT_a = np.ascontiguousarray(x.T).astype(bf)
    w1a, w1b = w1[:, :H], w1[:, H:]
    WA = w1a @ opw
    WB = w1b @ opw
    bv_full = ipb[2 * H:]
    node_bias = opw @ bv_full + opb
    bias_total = ((w1a + w1b) @ node_bias + b1).astype(np.float32)
    perm = np.argsort(w2v < 0, kind="stable")
    npos = int((w2v >= 0).sum())
    aw2 = np.abs(w2v)[perm]
    WA_t = np.ascontiguousarray((WA[perm] * aw2[:, None]).T).astype(bf)
    WB_t = np.ascontiguousarray((WB[perm] * aw2[:, None]).T).astype(bf)
    bias_perm = (bias_total[perm] * aw2).astype(np.float32)

    def wrap_idx(v):
        a = v.astype(np.int16).reshape(NGC, GCHUNK // 16, 16).transpose(0, 2, 1)
        blk = a.transpose(1, 0, 2).reshape(16, PC // 16)
        return np.ascontiguousarray(np.tile(blk, (8, 1)))

    in_maps = []
    for c in range(NCORES):
        h, half = c // 2, c % 2
        qoff = half * NQ
        sl = rel[c * PC:(c + 1) * PC]
        wqh = (ipw[DH * h:DH * (h + 1), :] * scale).T
        wkh = ipw[H + DH * h:H + DH * (h + 1), :].T
        wvh = ipw[2 * H + DH * h:2 * H + DH * (h + 1), :].T
        bqh = ipb[DH * h:DH * (h + 1)] * scale
        in_maps.append({
            "xT": xT_a,
            "xTq": np.ascontiguousarray(xT_a[:, qoff:qoff + NQ]),
            "wq_t": np.ascontiguousarray(np.tile(wqh, (1, RREP))).astype(bf),
            "wk_t": np.ascontiguousarray(np.tile(wkh, (1, RREP))).astype(bf),
            "wv_t": np.ascontiguousarray(wvh).astype(bf),
            "bq": np.tile(bqh, RREP).reshape(DH * RREP, 1).astype(np.float32),
            "wa_t": WA_t, "wb_t": WB_t,
            "bias_r": bias_perm.reshape(1, H).astype(bf),
            "srcw": wrap_idx(sl[:, 0]),
            "dstw": wrap_idx(sl[:, 1]),
        })
    return in_maps, npos


def kernel(**inputs):
    from concourse import bass_utils

    in_maps, npos = _prep_inputs(inputs)
    b2val = float(np.asarray(inputs["b2"], np.float32)[0])
    nc = _build(npos, b2val)
    res = bass_utils.run_bass_kernel_spmd(
        nc, in_maps, core_ids=list(range(NCORES)), trace=False)
    out = np.concatenate(
        [res.results[c]["preds"].T.reshape(-1) for c in range(NCORES)])
    return out.astype(np.float32)



# revision 10
# speedup vs baseline: 1.5261x; 1.5261x over previous
"""Trainium2 Bass kernel for nn_DroneRelationModel (8 NeuronCores).

Strategy:
  - Attention sharded (head, query-half) across 8 cores; scores computed
    transposed in PSUM (row-packed K=32 matmuls), ACT exp, AV matmul with a
    ones-column producing softmax denominators, normalize.
  - AllGather context pieces; every core folds out_proj+w1 into per-node
    tables A,B (|w2|-scaled, sign-permuted hidden, biases folded in).
  - Pair head sharded by pair slices: dma_gather rows of A,B per pair
    (natural layout), add + relu on DVE, sign-split segmented reduce gives
    w2-dot, sigmoid via tanh.
All heavy matmul inputs in bf16 (validated ~0.2% max rel err end to end).
"""
import numpy as np
import ml_dtypes

N, H, HEADS, P = 4096, 128, 4, 262144
DH = 32
NCORES = 8
NQ = 2048
QB = 1024
PC = P // NCORES
GCHUNK = 4096
NGC = PC // GCHUNK
RREP = 3


def _build(npos, b2val):
    import concourse.bass as bass
    import concourse.mybir as mybir
    import concourse.tile as tile
    from concourse import bacc

    F32 = mybir.dt.float32
    BF16 = mybir.dt.bfloat16
    I16 = mybir.dt.int16
    AF = mybir.ActivationFunctionType
    ALU = mybir.AluOpType

    nc = bacc.Bacc("TRN2", target_bir_lowering=False, debug=False, num_devices=NCORES,
                   num_swdge_queues=4)

    xT = nc.dram_tensor("xT", [H, N], BF16, kind="ExternalInput")
    xTq = nc.dram_tensor("xTq", [H, NQ], BF16, kind="ExternalInput")
    wq_t = nc.dram_tensor("wq_t", [H, DH * RREP], BF16, kind="ExternalInput")
    wk_t = nc.dram_tensor("wk_t", [H, DH * RREP], BF16, kind="ExternalInput")
    wv_t = nc.dram_tensor("wv_t", [H, DH], BF16, kind="ExternalInput")
    bq = nc.dram_tensor("bq", [DH * RREP, 1], F32, kind="ExternalInput")
    wa_t = nc.dram_tensor("wa_t", [H, H], BF16, kind="ExternalInput")
    wb_t = nc.dram_tensor("wb_t", [H, H], BF16, kind="ExternalInput")
    bias_r = nc.dram_tensor("bias_r", [1, H], BF16, kind="ExternalInput")
    srcw = nc.dram_tensor("srcw", [128, PC // 16], I16, kind="ExternalInput")
    dstw = nc.dram_tensor("dstw", [128, PC // 16], I16, kind="ExternalInput")
    preds = nc.dram_tensor("preds", [128, PC // 128], F32, kind="ExternalOutput")
    cc_in = nc.dram_tensor("cc_in", [DH, NQ], BF16)
    cc_out = nc.dram_tensor("cc_out", [NCORES, DH, NQ], BF16, addr_space="Shared")

    with tile.TileContext(nc) as tc:
        with tc.tile_pool(name="const", bufs=1) as cpool:
            def cload(name, dram, shape, dtype):
                t = cpool.tile(shape, dtype, tag=name)
                nc.sync.dma_start(out=t[...], in_=dram[...])
                return t

            xT_sb = cload("xT", xT, [H, N], BF16)
            xTq_sb = cload("xTq", xTq, [H, NQ], BF16)
            wq_sb = cload("wq", wq_t, [H, DH * RREP], BF16)
            wk_sb = cload("wk", wk_t, [H, DH * RREP], BF16)
            wv_sb = cload("wv", wv_t, [H, DH], BF16)
            bq_sb = cload("bq", bq, [DH * RREP, 1], F32)
            wa_sb = cload("wa", wa_t, [H, H], BF16)
            wb_sb = cload("wb", wb_t, [H, H], BF16)
            bias_sb = cload("bias", bias_r, [1, H], BF16)
            src_sb = cload("src", srcw, [128, PC // 16], I16)
            dst_sb = cload("dst", dstw, [128, PC // 16], I16)

            kT_sb = cpool.tile([DH * RREP, N], BF16, tag="kT")
            qT_sb = cpool.tile([DH * RREP, NQ], BF16, tag="qT")
            v_sb = cpool.tile([128, 33 * 32], BF16, tag="v")
            ctxn_sb = cpool.tile([DH, NQ], BF16, tag="ctxn")
            ones_sb = cpool.tile([1, H], BF16, tag="ones")
            nc.vector.memset(ones_sb[...], 1.0)

            with (
                tc.tile_pool(name="kqv_ps", bufs=2, space="PSUM") as kqv_ps,
                tc.tile_pool(name="v_ps", bufs=4, space="PSUM") as v_ps,
            ):
                for i in range(4):
                    ps = kqv_ps.tile([DH * RREP, QB], F32, tag="kq")
                    for j in range(2):
                        nc.tensor.matmul(ps[:, j * 512:(j + 1) * 512], wk_sb[...],
                                         xT_sb[:, i * QB + j * 512:i * QB + (j + 1) * 512],
                                         start=True, stop=True)
                    nc.vector.tensor_copy(kT_sb[:, i * QB:(i + 1) * QB], ps[...])
                for i in range(2):
                    ps = kqv_ps.tile([DH * RREP, QB], F32, tag="kq")
                    for j in range(2):
                        nc.tensor.matmul(ps[:, j * 512:(j + 1) * 512], wq_sb[...],
                                         xTq_sb[:, i * QB + j * 512:i * QB + (j + 1) * 512],
                                         start=True, stop=True)
                    nc.vector.tensor_scalar_add(qT_sb[:, i * QB:(i + 1) * QB],
                                                ps[...], bq_sb[...])
                nc.vector.memset(v_sb[...], 1.0)
                for kc in range(32):
                    ps = v_ps.tile([128, DH], F32, tag="v")
                    nc.tensor.matmul(ps[...], xT_sb[:, kc * 128:(kc + 1) * 128],
                                     wv_sb[...], start=True, stop=True)
                    nc.vector.tensor_copy(v_sb[:, kc * 33:kc * 33 + 32], ps[...])

            with (
                tc.tile_pool(name="s_ps", bufs=3, space="PSUM") as s_ps,
                tc.tile_pool(name="av_ps", bufs=1, space="PSUM") as av_psp,
                tc.tile_pool(name="es", bufs=6) as es_pool,
                tc.tile_pool(name="misc", bufs=2) as misc,
            ):
                for qb in range(2):
                    av_ps = av_psp.tile([128, QB], F32, tag="av")
                    es_tiles = {}
                    for kc in range(32):
                        r = kc % RREP
                        sp = s_ps.tile([128, QB], F32, tag="s")
                        for j in range(2):
                            nc.tensor.matmul(
                                sp[:, j * 512:(j + 1) * 512],
                                kT_sb[r * DH:(r + 1) * DH, kc * 128:(kc + 1) * 128],
                                qT_sb[r * DH:(r + 1) * DH,
                                      qb * QB + j * 512:qb * QB + (j + 1) * 512],
                                start=True, stop=True, tile_position=(r * DH, 0))
                        es = es_pool.tile([128, QB], BF16, tag="es")
                        nc.scalar.activation(es[...], sp[...], AF.Exp)
                        es_tiles[kc] = es
                        if kc % 2 == 1:
                            for t, kk in ((0, kc - 1), (1, kc)):
                                for j in range(2):
                                    nc.tensor.matmul(
                                        av_ps[t * 64:t * 64 + 33, j * 512:(j + 1) * 512],
                                        v_sb[:, kk * 33:(kk + 1) * 33],
                                        es_tiles[kk][:, j * 512:(j + 1) * 512],
                                        start=(kk < 2), stop=(kk >= 30),
                                        tile_position=(0, t * 64))
                            es_tiles.clear()
                    craw = misc.tile([33, QB], F32, tag="craw")
                    t1c = misc.tile([33, QB], F32, tag="t1c")
                    nc.vector.tensor_copy(t1c[...], av_ps[64:97, :])
                    nc.vector.tensor_add(craw[...], av_ps[0:33, :], t1c[...])
                    r_sb = misc.tile([1, QB], F32, tag="r")
                    nc.vector.reciprocal(r_sb[...], craw[32:33, :])
                    rbf = misc.tile([1, QB], BF16, tag="rbf")
                    nc.vector.tensor_copy(rbf[...], r_sb[...])
                    bc_ps = s_ps.tile([DH, QB], F32, tag="s")
                    for j in range(2):
                        nc.tensor.matmul(bc_ps[:, j * 512:(j + 1) * 512],
                                         ones_sb[:, 0:DH], rbf[:, j * 512:(j + 1) * 512],
                                         start=True, stop=True)
                    nc.vector.tensor_mul(ctxn_sb[:, qb * QB:(qb + 1) * QB],
                                         craw[0:32, :], bc_ps[...])

            nc.sync.dma_start(out=cc_in[...], in_=ctxn_sb[...])
            nc.gpsimd.collective_compute(
                "AllGather", ALU.bypass, replica_groups=[list(range(NCORES))],
                ins=[cc_in.ap()], outs=[cc_out.ap()])
            ctxT_sb = cpool.tile([H, N], BF16, tag="ctxT")
            for g in range(NCORES):
                hh, half = g // 2, g % 2
                nc.sync.dma_start(
                    out=ctxT_sb[hh * DH:(hh + 1) * DH, half * NQ:(half + 1) * NQ],
                    in_=cc_out[g, :, :])

            atab = cpool.tile([128, 32 * 128], BF16, tag="atab")
            btab = cpool.tile([128, 32 * 128], BF16, tag="btab")
            with tc.tile_pool(name="tab_ps", bufs=2, space="PSUM") as tab_ps:
                for t in range(32):
                    pa = tab_ps.tile([128, 128], F32, tag="ta")
                    pb = tab_ps.tile([128, 128], F32, tag="tb")
                    nc.tensor.matmul(pa[...], ones_sb[...], bias_sb[...],
                                     start=True, stop=False)
                    nc.tensor.matmul(pa[...], ctxT_sb[:, t * 128:(t + 1) * 128],
                                     wa_sb[...], start=False, stop=True)
                    nc.tensor.matmul(pb[...], ctxT_sb[:, t * 128:(t + 1) * 128],
                                     wb_sb[...], start=True, stop=True)
                    nc.vector.tensor_copy(atab[:, t * 128:(t + 1) * 128], pa[...])
                    nc.vector.tensor_copy(btab[:, t * 128:(t + 1) * 128], pb[...])
            atab_d = nc.dram_tensor("atab_d", [N, H], BF16)
            btab_d = nc.dram_tensor("btab_d", [N, H], BF16)
            nc.sync.dma_start(out=atab_d.ap().rearrange("(c p) d -> p c d", p=128),
                              in_=atab[...].rearrange("p (c d) -> p c d", d=128))
            nc.sync.dma_start(out=btab_d.ap().rearrange("(c p) d -> p c d", p=128),
                              in_=btab[...].rearrange("p (c d) -> p c d", d=128))

            with (
                tc.tile_pool(name="gat", bufs=2) as gpool,
                tc.tile_pool(name="hseq", bufs=2) as hpool,
                tc.tile_pool(name="sig", bufs=1) as sig_pool,
            ):
                NB = GCHUNK // 128
                lsp = sig_pool.tile([128, PC // 128], F32, tag="lsp")
                lsn = sig_pool.tile([128, PC // 128], F32, tag="lsn")
                iw = GCHUNK // 16
                for g in range(NGC):
                    sg = gpool.tile([128, NB, 128], BF16, tag="sg")
                    dg = gpool.tile([128, NB, 128], BF16, tag="dg")
                    nc.gpsimd.dma_gather(
                        sg[...], atab_d[...], src_sb[:, g * iw:(g + 1) * iw],
                        num_idxs=GCHUNK, num_idxs_reg=GCHUNK, elem_size=128,
                        transpose=False, single_packet=False,
                        queue_num=1 + (2 * g) % 3)
                    nc.gpsimd.dma_gather(
                        dg[...], btab_d[...], dst_sb[:, g * iw:(g + 1) * iw],
                        num_idxs=GCHUNK, num_idxs_reg=GCHUNK, elem_size=128,
                        transpose=False, single_packet=False,
                        queue_num=1 + (2 * g + 1) % 3)
                    hs = hpool.tile([128, NB, 128], BF16, tag="hs")
                    nc.vector.tensor_add(hs[...].rearrange("p b d -> p (b d)"),
                                         sg[...].rearrange("p b d -> p (b d)"),
                                         dg[...].rearrange("p b d -> p (b d)"))
                    hr = hpool.tile([128, NB, 128], BF16, tag="hr")
                    nc.vector.tensor_scalar_max(hr[...].rearrange("p b d -> p (b d)"),
                                                hs[...].rearrange("p b d -> p (b d)"), 0.0)
                    nc.vector.tensor_reduce(
                        lsp[:, g * NB:(g + 1) * NB].rearrange("p b -> p b ()"),
                        hr[:, :, 0:npos], op=ALU.add, axis=mybir.AxisListType.X)
                    nc.vector.tensor_reduce(
                        lsn[:, g * NB:(g + 1) * NB].rearrange("p b -> p b ()"),
                        hr[:, :, npos:H], op=ALU.add, axis=mybir.AxisListType.X)
                logit = sig_pool.tile([128, PC // 128], F32, tag="logit")
                nc.vector.tensor_sub(logit[...], lsp[...], lsn[...])
                sig_sb = sig_pool.tile([128, PC // 128], F32, tag="sig")
                nc.scalar.activation(sig_sb[...], logit[...], AF.Tanh,
                                     scale=0.5, bias=0.5 * b2val)
                nc.vector.tensor_scalar(sig_sb[...], sig_sb[...], 0.5, 0.5,
                                        op0=ALU.mult, op1=ALU.add)
                nc.sync.dma_start(out=preds[...], in_=sig_sb[...])
    nc.compile()
    return nc


def _prep_inputs(inputs):
    x = np.asarray(inputs["context_embeddings"], np.float32)
    ipw = np.asarray(inputs["in_proj_w"], np.float32)
    ipb = np.asarray(inputs["in_proj_b"], np.float32)
    opw = np.asarray(inputs["out_proj_w"], np.float32)
    opb = np.asarray(inputs["out_proj_b"], np.float32)
    w1 = np.asarray(inputs["w1"], np.float32)
    b1 = np.asarray(inputs["b1"], np.float32)
    w2v = np.asarray(inputs["w2"], np.float32)[0]
    rel = np.asarray(inputs["relationships"])

    bf = ml_dtypes.bfloat16
    scale = 1.0 / np.sqrt(DH)
    xT_a = np.ascontiguousarray(x.T).astype(bf)
    w1a, w1b = w1[:, :H], w1[:, H:]
    WA = w1a @ opw
    WB = w1b @ opw
    bv_full = ipb[2 * H:]
    node_bias = opw @ bv_full + opb
    bias_total = ((w1a + w1b) @ node_bias + b1).astype(np.float32)
    perm = np.argsort(w2v < 0, kind="stable")
    npos = int((w2v >= 0).sum())
    aw2 = np.abs(w2v)[perm]
    WA_t = np.ascontiguousarray((WA[perm] * aw2[:, None]).T).astype(bf)
    WB_t = np.ascontiguousarray((WB[perm] * aw2[:, None]).T).astype(bf)
    bias_perm = (bias_total[perm] * aw2).astype(np.float32)

    def wrap_idx(v):
        a = v.astype(np.int16).reshape(NGC, GCHUNK // 16, 16).transpose(0, 2, 1)
        blk = a.transpose(1, 0, 2).reshape(16, PC // 16)
        return np.ascontiguousarray(np.tile(blk, (8, 1)))

    in_maps = []
    for c in range(NCORES):
        h, half = c // 2, c % 2
        qoff = half * NQ
        sl = rel[c * PC:(c + 1) * PC]
        wqh = (ipw[DH * h:DH * (h + 1), :] * scale).T
        wkh = ipw[H + DH * h:H + DH * (h + 1), :].T
        wvh = ipw[2 * H + DH * h:2 * H + DH * (h + 1), :].T
        bqh = ipb[DH * h:DH * (h + 1)] * scale
        in_maps.append({
            "xT": xT_a,
            "xTq": np.ascontiguousarray(xT_a[:, qoff:qoff + NQ]),
            "wq_t": np.ascontiguousarray(np.tile(wqh, (1, RREP))).astype(bf),
            "wk_t": np.ascontiguousarray(np.tile(wkh, (1, RREP))).astype(bf),
            "wv_t": np.ascontiguousarray(wvh).astype(bf),
            "bq": np.tile(bqh, RREP).reshape(DH * RREP, 1).astype(np.float32),
            "wa_t": WA_t, "wb_t": WB_t,
            "bias_r": bias_perm.reshape(1, H).astype(bf),
            "srcw": wrap_idx(sl[:, 0]),
            "dstw": wrap_idx(sl[:, 1]),
        })
    return in_maps, npos


def kernel(**inputs):
    from concourse import bass_utils

    in_maps, npos = _prep_inputs(inputs)
    b2val = float(np.asarray(inputs["b2"], np.float32)[0])
    nc = _build(npos, b2val)
    res = bass_utils.run_bass_kernel_spmd(
        nc, in_maps, core_ids=list(range(NCORES)), trace=False)
    out = np.concatenate(
        [res.results[c]["preds"].T.reshape(-1) for c in range(NCORES)])
    return out.astype(np.float32)

